# revision 27
# baseline (speedup 1.0000x reference)
"""Trainium2 Bass kernel for nn_Encoder_88983132439258 (GNN message passing).

Strategy (8 NeuronCores, data-parallel over destination nodes = graphs):
  - Feature-major z-tables live in SBUF: T [128, 65536] bf16 where partition
    p holds feature p%32 of all 65536 nodes (4 replicas of z^T [32, N]).
  - Edge gather runs on GPSIMD ap_gather (d=2 "pair gather"): index
    src//2 (int16) fetches both nodes of a pair; the wrong pair element is
    killed by a host-built weight mask w2 (padding slots have w=0 too).
    One instruction gathers 4 dst-streams x 8192 slots.
  - Aggregation: one DVE tensor_tensor (m = g*w2) and one DVE tensor_reduce
    over the 2*PAD slot elements per dst -> agg [4s x 32f, 128 dst] f32.
  - Stream s of chunk c owns dsts [s*2048 + c*128, +128) so all stores are
    plain 2D APs (partition dim = (s,f) order matches DRAM row order).
  - ELU in feature-major (bias = per-partition), next-layer z via one
    block-diagonal (4x W) 128x128 matmul.
  - Tables are AllGathered ([128,2048] per core -> [1024,2048]) and
    reloaded into SBUF with 16 reshuffling DMAs.
  - FC head: layer-3 output is exchanged with AllToAll so core j owns
    H^T rows for its node-window of all 16 graphs; one dma_start_transpose
    forms all 128 lhsT chunks; 128 matmuls accumulate FC1 in PSUM;
    partials AllReduced; ELU+FC2 replicated on every core.
"""

import numpy as np
import ml_dtypes

import concourse.bacc as bacc
import concourse.mybir as mybir
import concourse.tile as tile
import concourse.bass as bass
from concourse import bass_utils

F32 = mybir.dt.float32
BF16 = mybir.dt.bfloat16
I16 = mybir.dt.int16

N = 65536
NODES_PER = 4096
N_GRAPHS = 16
FEAT_IN = 16
HID = 32
FC_HID = 256
LATENT = 64
NC = 8                 # cores
OWN = N // NC          # 8192 dsts per core
P = 128
NI = 8192              # gather indices per 16-partition group per chunk

_prog_cache = {}


def _build_program(PAD, PADS):
    DPS = NI // PAD        # dsts per stream per chunk (128 for PAD=64)
    SBLK = OWN // 4        # dsts per stream block (2048)
    NCHUNK = SBLK // DPS   # chunks per core (16)
    KFC = 16384            # FC1 contraction elems per core
    NCFC = KFC // P        # 128 matmul chunks

    nc = bacc.Bacc("TRN2", target_bir_lowering=False, debug=False,
                   num_devices=NC)

    # ---- I/O ----
    tabf_in = nc.dram_tensor("tabf", [HID, N], BF16, kind="ExternalInput")
    idx_in = nc.dram_tensor("idxp", [NCHUNK, P, NI // 16], I16,
                            kind="ExternalInput")
    w2_in = nc.dram_tensor("w2d", [NCHUNK, 4, 2 * NI], BF16,
                           kind="ExternalInput")
    SUMI = sum(P * pc // 16 for pc in PADS)
    SUMW = sum(2 * P * pc for pc in PADS)
    idxs_in = nc.dram_tensor("idxsF", [P, SUMI], I16, kind="ExternalInput")
    w2s_in = nc.dram_tensor("w2sF", [4, SUMW], BF16, kind="ExternalInput")
    wbd_in = nc.dram_tensor("wbd", [2, P, P], BF16, kind="ExternalInput")
    bst_in = nc.dram_tensor("bst", [3, P], F32, kind="ExternalInput")
    idf_in = nc.dram_tensor("identf", [N_GRAPHS, N_GRAPHS], F32,
                            kind="ExternalInput")
    wfc1_in = nc.dram_tensor("wfc1s", [KFC, FC_HID], BF16,
                             kind="ExternalInput")
    wfc2_in = nc.dram_tensor("wfc2s", [2 * P, LATENT], F32,
                             kind="ExternalInput")
    bfc1_in = nc.dram_tensor("bfc1t", [N_GRAPHS, FC_HID], F32,
                             kind="ExternalInput")
    bfc2_in = nc.dram_tensor("bfc2t", [N_GRAPHS, LATENT], F32,
                             kind="ExternalInput")
    out = nc.dram_tensor("out", [N_GRAPHS, LATENT], F32,
                         kind="ExternalOutput")

    # ---- internal DRAM ----
    # cinF[p, c*DPS+d]: partition-major local table chunk
    cinF = nc.dram_tensor("cinF", [P, NCHUNK * DPS], BF16)
    tabAG = nc.dram_tensor("tabAG", [NC * P, NCHUNK * DPS], BF16,
                           addr_space="Shared")
    # a2a block content: [(s f) p, q, d] per receiving core
    a2a_in = nc.dram_tensor("a2ain", [NC, P * 2 * DPS], BF16)
    a2a_out = nc.dram_tensor("a2aout", [NC, P * 2 * DPS], BF16)
    arin = nc.dram_tensor("arin", [N_GRAPHS, FC_HID], F32)
    arout = nc.dram_tensor("arout", [N_GRAPHS, FC_HID], F32,
                           addr_space="Shared")

    groups = [list(range(NC))]

    with tile.TileContext(nc) as tc:
        with tc.tile_pool(name="cst", bufs=1) as cst, \
             tc.tile_pool(name="big", bufs=1) as big, \
             tc.tile_pool(name="psA", bufs=1, space="PSUM") as psA, \
             tc.tile_pool(name="ps1", bufs=1, space="PSUM") as ps1, \
             tc.tile_pool(name="psfc", bufs=1, space="PSUM") as psfc:

            # ---- constants ----
            wbd_t = [cst.tile([P, P], BF16, tag=f"wbd{i}", name=f"wbd{i}")
                     for i in range(2)]
            for i in range(2):
                nc.sync.dma_start(out=wbd_t[i][:], in_=wbd_in[i])
            bst_t = [cst.tile([P, 1], F32, tag=f"bst{i}", name=f"bst{i}")
                     for i in range(3)]
            for i in range(3):
                nc.sync.dma_start(out=bst_t[i][:],
                                  in_=bst_in[i].rearrange("(p o) -> p o", o=1))
            idf_t = cst.tile([N_GRAPHS, N_GRAPHS], F32, tag="idf")
            nc.sync.dma_start(out=idf_t[:], in_=idf_in[:, :])
            wfc2_t = cst.tile([P, 2, LATENT], F32, tag="wfc2")
            nc.sync.dma_start(
                out=wfc2_t[:],
                in_=wfc2_in.ap().rearrange("(h p) o -> p h o", p=P))
            bfc1_t = cst.tile([N_GRAPHS, FC_HID], F32, tag="bfc1")
            nc.sync.dma_start(out=bfc1_t[:], in_=bfc1_in[:, :])
            bfc2_t = cst.tile([N_GRAPHS, LATENT], F32, tag="bfc2")
            nc.sync.dma_start(out=bfc2_t[:], in_=bfc2_in[:, :])

            # ---- SBUF table (4 replicas of z^T), one broadcast DMA ----
            T = big.tile([P, N], BF16, tag="T")
            for hf in range(2):
                nc.sync.dma_start(
                    out=T[:, hf * (N // 2):(hf + 1) * (N // 2)],
                    in_=tabf_in.ap()[:, hf * (N // 2):(hf + 1) * (N // 2)]
                    .rearrange("(o f) n -> o f n", o=1)
                    .to_broadcast((4, HID, N // 2)))

            g = big.tile([P, 2 * NI], BF16, tag="g")
            w2 = big.tile([P, 2 * NI], BF16, tag="w2")
            it = big.tile([P, 4 * (NI // 16)], I16, tag="it")

            WID = NCHUNK * DPS     # 2048 wide-batch columns (full layer)

            def conv_layer(li):
                pA = psA.tile([P, WID], F32, tag="pA")
                io = wo = 0
                for c in range(NCHUNK):
                    if li < 2:
                        # sorted geometry: per-chunk PAD
                        pc = PADS[c]
                        nic = DPS * pc
                        nc.sync.dma_start(
                            out=it[:, :nic // 16],
                            in_=idxs_in.ap()[:, io:io + nic // 16])
                        itv = it[:, :nic // 16]
                        nc.sync.dma_start(
                            out=w2[:, :2 * nic],
                            in_=w2s_in.ap()[:, wo:wo + 2 * nic]
                            .rearrange("s (o n) -> s o n", o=1)
                            .to_broadcast((4, HID, 2 * nic)))
                        io += nic // 16
                        wo += 2 * nic
                    else:
                        pc = PAD
                        nic = NI
                        if c % 4 == 0:
                            nc.sync.dma_start(
                                out=it[:].rearrange("p (q n) -> p q n", q=4),
                                in_=idx_in.ap()[c:c + 4].rearrange(
                                    "q p n -> p q n"))
                        itv = it[:, (c % 4) * (NI // 16):
                                 (c % 4 + 1) * (NI // 16)]
                        nc.sync.dma_start(
                            out=w2[:],
                            in_=w2_in.ap()[c].rearrange(
                                "s (o n) -> s o n", o=1)
                            .to_broadcast((4, HID, 2 * NI)))
                    nc.gpsimd.ap_gather(
                        out_ap=g[:, :2 * nic], in_ap=T[:], idxs_ap=itv,
                        channels=P, num_elems=N // 2, d=2, num_idxs=nic)
                    nc.vector.tensor_tensor(
                        out=g[:, :2 * nic], in0=g[:, :2 * nic],
                        in1=w2[:, :2 * nic],
                        op=mybir.AluOpType.mult)
                    nc.vector.tensor_reduce(
                        out=pA[:, c * DPS:(c + 1) * DPS],
                        in_=g[:, :2 * nic].rearrange("p (d e) -> p d e",
                                                     e=2 * pc),
                        axis=mybir.AxisListType.X,
                        op=mybir.AluOpType.add)
                # wide ELU(agg + b) on [128, WID]; scratch = w2 views
                rlw = w2[:, 0:WID]
                mnw = w2[:, WID:3 * WID].bitcast(F32)
                exw = w2[:, 3 * WID:5 * WID].bitcast(F32)
                hw = w2[:, 5 * WID:6 * WID]
                tnw = w2[:, 6 * WID:7 * WID]
                nc.scalar.activation(rlw, pA[:],
                                     mybir.ActivationFunctionType.Relu,
                                     bias=bst_t[li][:])
                nc.vector.scalar_tensor_tensor(
                    out=mnw, in0=pA[:], scalar=bst_t[li][:],
                    in1=rlw, op0=mybir.AluOpType.add,
                    op1=mybir.AluOpType.subtract)
                nc.scalar.activation(exw, mnw,
                                     mybir.ActivationFunctionType.Exp)
                nc.vector.scalar_tensor_tensor(
                    out=hw, in0=rlw, scalar=-1.0, in1=exw,
                    op0=mybir.AluOpType.add, op1=mybir.AluOpType.add)
                if li < 2:
                    for j2 in range(4):
                        pz = ps1.tile([P, WID // 4], F32, tag="pz")
                        nc.tensor.matmul(
                            out=pz[:], lhsT=wbd_t[li][:],
                            rhs=hw.rearrange(
                                "p (j n) -> p j n", j=4)[:, j2],
                            start=True, stop=True)
                        nc.scalar.copy(
                            tnw.rearrange("p (j n) -> p j n",
                                          j=4)[:, j2], pz[:])
                    nc.sync.dma_start(out=cinF.ap(), in_=tnw)
                else:
                    for q in range(2):
                        nc.sync.dma_start(
                            out=a2a_in.ap().rearrange(
                                "i (p q d) -> q i p d",
                                p=P, q=2)[q].rearrange("i p d -> p i d"),
                            in_=hw.rearrange(
                                "p (i q d) -> p q i d",
                                q=2, d=DPS)[:, q])

                if li < 2:
                    nc.gpsimd.collective_compute(
                        "AllGather", mybir.AluOpType.bypass,
                        replica_groups=groups,
                        ins=[cinF.ap().opt()], outs=[tabAG.ap().opt()])
                    for r in range(4):
                        for s in range(4):
                            nc.sync.dma_start(
                                out=T[HID * r:HID * (r + 1), :].rearrange(
                                    "f (k s cd) -> f k s cd",
                                    k=NC, s=4)[:, :, s],
                                in_=tabAG.ap().rearrange(
                                    "(k s f) cd -> f k s cd",
                                    s=4, f=HID)[:, :, s])

            conv_layer(0)
            conv_layer(1)
            conv_layer(2)

            nc.gpsimd.collective_compute(
                "AllToAll", mybir.AluOpType.bypass, replica_groups=groups,
                ins=[a2a_in.ap().opt()], outs=[a2a_out.ap().opt()])

            # ---- FC head ----
            wfc_full = big.tile([P, N], BF16, tag="T")  # reuse T buffer
            wfc = wfc_full[:, :NCFC * FC_HID].rearrange(
                "p (c o) -> p c o", o=FC_HID)
            nc.sync.dma_start(
                out=wfc,
                in_=wfc1_in.ap().rearrange("(c p) o -> p c o", p=P))
            lhsT = cst.tile([P, NCFC, N_GRAPHS], BF16, tag="lhsT")
            nc.sync.dma_start_transpose(
                out=lhsT[:],
                in_=a2a_out.ap().rearrange("j (gl x) -> (j gl) x", gl=2))
            pfc = psfc.tile([N_GRAPHS, FC_HID], F32, tag="pfc")
            for c in range(NCFC):
                nc.tensor.matmul(out=pfc[:], lhsT=lhsT[:, c, :],
                                 rhs=wfc[:, c, :],
                                 start=(c == 0), stop=(c == NCFC - 1))
            # FC scratch: f32 views of the (dead) g buffer
            part = g[:N_GRAPHS, 0:512].bitcast(F32)
            u2 = g[:N_GRAPHS, 512:1024].bitcast(F32)
            rl2 = g[:N_GRAPHS, 1024:1536].bitcast(F32)
            mn2 = g[:N_GRAPHS, 1536:2048].bitcast(F32)
            ex2 = g[:N_GRAPHS, 2048:2560].bitcast(F32)
            fin = g[:N_GRAPHS, 2624:2752].bitcast(F32)
            nc.vector.tensor_copy(part, pfc[:])
            nc.sync.dma_start(out=arin.ap(), in_=part)
            nc.gpsimd.collective_compute(
                "AllReduce", mybir.AluOpType.add, replica_groups=groups,
                ins=[arin.ap().opt()], outs=[arout.ap().opt()])
            nc.sync.dma_start(out=part, in_=arout.ap())
            nc.vector.tensor_tensor(out=u2, in0=part, in1=bfc1_t[:],
                                    op=mybir.AluOpType.add)
            nc.scalar.activation(rl2, u2,
                                 mybir.ActivationFunctionType.Relu)
            nc.vector.scalar_tensor_tensor(
                out=mn2, in0=u2, scalar=0.0, in1=rl2,
                op0=mybir.AluOpType.add, op1=mybir.AluOpType.subtract)
            nc.scalar.activation(ex2, mn2,
                                 mybir.ActivationFunctionType.Exp)
            fcm = u2
            nc.vector.scalar_tensor_tensor(
                out=fcm, in0=rl2, scalar=-1.0, in1=ex2,
                op0=mybir.AluOpType.add, op1=mybir.AluOpType.add)
            pP = psfc.tile([N_GRAPHS, LATENT], F32, tag="pP")
            for hh in range(2):
                tp = ps1.tile([P, N_GRAPHS], F32, tag="tp",
                              name=f"tp{hh}")
                nc.tensor.transpose(out=tp[:],
                                    in_=fcm[:, hh * P:(hh + 1) * P],
                                    identity=idf_t[:])
                tcp = g[:, 2560 + hh * 32:2560 + (hh + 1) * 32].bitcast(F32)
                nc.vector.tensor_copy(tcp, tp[:])
                nc.tensor.matmul(out=pP[:], lhsT=tcp,
                                 rhs=wfc2_t[:, hh, :],
                                 start=(hh == 0), stop=(hh == 1))
            nc.vector.tensor_tensor(out=fin, in0=pP[:], in1=bfc2_t[:],
                                    op=mybir.AluOpType.add)
            nc.sync.dma_start(out=out.ap(), in_=fin)

    nc.compile()
    return nc


def _host_prep(inputs):
    x = np.asarray(inputs["x"], np.float32)
    ei = np.asarray(inputs["edge_index"])
    w = np.asarray(inputs["edge_attr"], np.float32)
    W1 = np.asarray(inputs["W1"], np.float32)
    b1 = np.asarray(inputs["b1"], np.float32)
    W2 = np.asarray(inputs["W2"], np.float32)
    b2 = np.asarray(inputs["b2"], np.float32)
    W3 = np.asarray(inputs["W3"], np.float32)
    b3 = np.asarray(inputs["b3"], np.float32)
    Wfc1 = np.asarray(inputs["Wfc1"], np.float32)
    bfc1 = np.asarray(inputs["bfc1"], np.float32)
    Wfc2 = np.asarray(inputs["Wfc2"], np.float32)
    bfc2 = np.asarray(inputs["bfc2"], np.float32)

    src = ei[0].astype(np.int64)
    dst = ei[1].astype(np.int64)
    E = src.shape[0]

    order = np.argsort(dst, kind="stable")
    d_s = dst[order]
    s_s = src[order]
    w_s = w[order]
    deg = np.bincount(d_s, minlength=N)
    PAD = 8
    while PAD < int(deg.max()):
        PAD *= 2
    starts = np.zeros(N + 1, np.int64)
    np.cumsum(deg, out=starts[1:])
    pos = np.arange(E, dtype=np.int64) - starts[d_s]

    slot_idx = np.zeros((N, PAD), dtype=np.int32)
    slot_w = np.zeros((N, PAD), dtype=np.float32)
    slot_idx[d_s, pos] = s_s.astype(np.int32)
    slot_w[d_s, pos] = w_s

    assert PAD == 64, f"kernel geometry assumes PAD=64, got {PAD}"
    DPS = NI // PAD
    NCHUNK = (OWN // 4) // DPS

    # --- degree-sorted table permutation (per core), shared chunk PADs ---
    # table position k*OWN + s*2048 + p holds node k*OWN + order_k[p*4+s]
    orders = np.zeros((NC, OWN), np.int64)
    posg = np.zeros(N, np.int64)          # node -> table position
    PADS = np.zeros(NCHUNK, np.int64)
    for k in range(NC):
        dk = deg[k * OWN:(k + 1) * OWN]
        order = np.argsort(-dk, kind="stable")
        orders[k] = order
        r = np.arange(OWN)
        posg[k * OWN + order] = k * OWN + (r % 4) * 2048 + r // 4
        cm = dk[order].reshape(NCHUNK, DPS, 4).max(axis=(1, 2))
        PADS = np.maximum(PADS, cm)
    PADS = tuple(int(v) for v in np.maximum(PADS, 4))

    pos_slot = posg[slot_idx]                  # [N, PAD] table positions
    pair = (pos_slot >> 1).astype(np.int16)
    parity = (pos_slot & 1).astype(np.int8)

    # layers 1-2: sorted geometry, flat-packed per chunk
    SUMI = sum(P * pc // 16 for pc in PADS)
    SUMW = sum(2 * P * pc for pc in PADS)
    idxsF = np.zeros((NC, P, SUMI), np.int16)
    w2sF = np.zeros((NC, 4, SUMW), np.float32)
    for k in range(NC):
        io = wo = 0
        for c, pc in enumerate(PADS):
            nic = DPS * pc
            dl = orders[k][np.arange(c * DPS, (c + 1) * DPS)[:, None] * 4 +
                           np.arange(4)[None, :]]           # [DPS, 4]
            gids = k * OWN + dl                             # global dst ids
            pr = pair[gids][:, :, :pc].transpose(1, 0, 2).reshape(4, nic)
            pa = parity[gids][:, :, :pc].transpose(1, 0, 2).reshape(4, nic)
            wv = slot_w[gids][:, :, :pc].transpose(1, 0, 2).reshape(4, nic)
            iw1 = pr.reshape(4, nic // 16, 16).transpose(0, 2, 1)
            idxsF[k, :, io:io + nic // 16] = np.repeat(
                iw1, 2, axis=0).reshape(P, nic // 16)
            o = np.zeros((4, nic, 2), np.float32)
            np.put_along_axis(o, np.ascontiguousarray(pa)[..., None]
                              .astype(np.int64),
                              np.ascontiguousarray(wv)[..., None], axis=2)
            w2sF[k, :, wo:wo + 2 * nic] = o.reshape(4, 2 * nic)
            io += nic // 16
            wo += 2 * nic
    w2sF = w2sF.astype(ml_dtypes.bfloat16)

    # layer 3: natural dst geometry (pi-space slot indices)
    pr_c = pair.reshape(NC, 4, NCHUNK, DPS * PAD).transpose(0, 2, 1, 3)
    pa_c = parity.reshape(NC, 4, NCHUNK, DPS * PAD).transpose(0, 2, 1, 3)
    w_c = slot_w.reshape(NC, 4, NCHUNK, DPS * PAD).transpose(0, 2, 1, 3)

    # wrapped indices: [NC, NCHUNK, 128, NI//16]; i = col*16 + part
    iw = pr_c.reshape(NC, NCHUNK, 4, NI // 16, 16)
    iw = np.ascontiguousarray(iw.transpose(0, 1, 2, 4, 3))
    idx_wrapped = np.repeat(iw, 2, axis=2).reshape(NC, NCHUNK, P, NI // 16)

    # w2: [NC, NCHUNK, 4, 2*NI]  col = i*2 + j, kill wrong pair element
    w2d = np.zeros((NC, NCHUNK, 4, NI, 2), np.float32)
    np.put_along_axis(w2d, np.ascontiguousarray(pa_c)[..., None]
                      .astype(np.int64), np.ascontiguousarray(w_c)[..., None],
                      axis=4)
    w2d = w2d.reshape(NC, NCHUNK, 4, 2 * NI).astype(ml_dtypes.bfloat16)

    # pi-permuted layer-1 table: tabf[:, n] = z1[perm[n]]
    permg = np.zeros(N, np.int64)
    permg[posg] = np.arange(N)
    tabf = np.ascontiguousarray((x @ W1)[permg].T).astype(ml_dtypes.bfloat16)

    def blockdiag(W):
        out_ = np.zeros((P, P), np.float32)
        for t in range(4):
            out_[t * HID:(t + 1) * HID, t * HID:(t + 1) * HID] = W
        return out_.astype(ml_dtypes.bfloat16)

    wbd = np.stack([blockdiag(W2), blockdiag(W3)])
    bst = np.stack([np.tile(b1, 4), np.tile(b2, 4),
                    np.tile(b3, 4)]).astype(np.float32)
    identf = np.eye(N_GRAPHS, dtype=np.float32)

    # FC1 per-core slice: k' = sp*8192 + f*256 + q*128 + d maps to global
    # node ng = sp*2048 + (2j+q)*128 + d of every graph, feature f.
    # Wfc1 [131072, 256] -> [sp 2, m 32, d 128, f 32, o]
    wfc1_r = Wfc1.reshape(2, 16, DPS, HID, FC_HID)   # [sp, m, d, f, o]
    in_maps = []
    for k in range(NC):
        # receiver k gets senders' chunks c = 2k, 2k+1 -> m = 2k + q
        wj = wfc1_r[:, 2 * k:2 * k + 2]              # [sp, q 2, d, f, o]
        wj = np.ascontiguousarray(wj.transpose(0, 3, 1, 2, 4)).reshape(
            KFC_HOST, FC_HID).astype(ml_dtypes.bfloat16)
        in_maps.append({
            "tabf": tabf,
            "idxp": idx_wrapped[k],
            "w2d": w2d[k],
            "idxsF": idxsF[k],
            "w2sF": w2sF[k],
            "wbd": wbd,
            "bst": bst,
            "identf": identf,
            "wfc1s": wj,
            "wfc2s": np.ascontiguousarray(Wfc2.astype(np.float32)),
            "bfc1t": np.tile(bfc1, (N_GRAPHS, 1)),
            "bfc2t": np.tile(bfc2, (N_GRAPHS, 1)),
        })
    return PAD, PADS, in_maps


KFC_HOST = 16384


def kernel(**inputs):
    PAD, PADS, in_maps = _host_prep(inputs)
    key = (PAD, PADS)
    if key not in _prog_cache:
        _prog_cache[key] = _build_program(PAD, PADS)
    nc = _prog_cache[key]
    res = bass_utils.run_bass_kernel_spmd(nc, in_maps,
                                          core_ids=list(range(NC)))
    return np.asarray(res.results[0]["out"], np.float32)
